# revision 19
# baseline (speedup 1.0000x reference)
"""Causal single-head attention on 8 Trainium2 NeuronCores (Bass/Tile), v4.

Problem: x[4,2048,1024] fp32, Wq/Wk/Wv[1024,1024];
  q,k,v = x@W.T ; S = q@k.T/sqrt(d) ; causal softmax ; out = P@v.

Sharding (balanced causal): core c -> batch b=c//2, half h=c%2. Each core
owns 8 query blocks of 128 interleaved so causal work balances:
  h=0: global q-blocks [0,3,4,7,8,11,12,15]   (key extents 1,4,5,8,9,12,13,16)
  h=1: global q-blocks [1,2,5,6,9,10,13,14]   (extents 2,3,6,7,10,11,14,15)
Each core projects Q/K/V only for its own 1024 rows; K/V halves are exchanged
with a pairwise AllGather (groups [[0,1],[2,3],[4,5],[6,7]]), giving both
cores the full K/V in "gathered" key order [A-blocks asc | B-blocks asc].

Attention slot j (local q-block j, processed j=7..0) computes scores only
over the causally-needed prefix: the first j+1 gathered tiles of EACH half
(<=1 padded tile per slot; 72 of 128 tiles computed per core). Only the
last tile of each half can be partial/masked; the mask is built data-driven
from kpos/qpos (SPMD-uniform program, per-core data).

All SBUF-resident operands use block-interleaved [128, n*1024] layouts so
every bulk transfer is a single large DMA (the HW cost is dominated by
per-DMA overhead at these sizes): x/W/Q/K-stage are one tile each; the
gathered K^T and V are [128, 16384] tiles split [A-half | B-half], each
half ob-/tile-major so score and AV moving slices stay contiguous.

Numerics: bf16 operands with fp32 PSUM accumulation; softmax without
max-subtraction (scores are O(+-6)); denominator via activation accum_out;
P@V via PE-transposed P tiles; bf16 output upcast on host. End-to-end rel
err ~5e-3 (gate 2e-2).
"""

import sys

sys.path.insert(0, "/opt/trn_rl_repo")

from contextlib import ExitStack

import numpy as np
import ml_dtypes

import concourse.bass as bass  # noqa: F401
from concourse import bacc
import concourse.mybir as mybir
import concourse.tile as tile
from concourse.bass_utils import run_bass_kernel_spmd

F32 = mybir.dt.float32
BF16 = mybir.dt.bfloat16

B, N, D = 4, 2048, 1024
P = 128
NQ = 1024        # local queries per core
NB = 8           # local q-blocks per core
W8 = 8 * 1024    # interleaved big-tile width
ASC_A = [0, 3, 4, 7, 8, 11, 12, 15]
ASC_B = [1, 2, 5, 6, 9, 10, 13, 14]
GATHERED = ASC_A + ASC_B
MASK_VAL = -1.0e30
GROUPS = [[0, 1], [2, 3], [4, 5], [6, 7]]

_CACHE = {}


def _vis_chunks(w):
    """Split visible width w (multiple of 128, <=896) into chunks <=512,
    avoiding a trailing 128-wide chunk where possible."""
    out, o, rem = [], 0, w
    while rem > 0:
        c = 512 if rem >= 512 else rem
        if 512 < rem < 768:
            c = rem - 256
        out.append((o, o + c))
        o += c
        rem -= c
    return out


def _build_program(iters=1, phase="full"):
    nc = bacc.Bacc("TRN2", target_bir_lowering=False, debug=False, num_devices=8)
    xT = nc.dram_tensor("xT", [P, W8], BF16, kind="ExternalInput").ap()
    wqT = nc.dram_tensor("wqT", [P, W8], BF16, kind="ExternalInput").ap()
    wkT = nc.dram_tensor("wkT", [P, W8], BF16, kind="ExternalInput").ap()
    wvT = nc.dram_tensor("wvT", [P, W8], BF16, kind="ExternalInput").ap()
    kposd = nc.dram_tensor("kpos", [P, N], F32, kind="ExternalInput").ap()
    qposd = nc.dram_tensor("qpos", [P, NB], F32, kind="ExternalInput").ap()
    identd = nc.dram_tensor("ident", [P, P], BF16, kind="ExternalInput").ap()
    out = nc.dram_tensor("out", [NQ, D], BF16, kind="ExternalOutput").ap()

    with tile.TileContext(nc) as tc, ExitStack() as cstack:
        # loop-invariant constants (loaded once per launch, like the
        # single-shot kernel)
        const = cstack.enter_context(tc.tile_pool(name="const", bufs=1))
        ident = const.tile([P, P], BF16, tag="ident")
        nc.sync.dma_start(ident[:], identd[:, :])
        kpos = const.tile([P, N], F32, tag="kpos")
        nc.sync.dma_start(kpos[:], kposd[:, :])
        qpos = const.tile([P, NB], F32, tag="qpos")
        nc.sync.dma_start(qpos[:], qposd[:, :])

        if iters == 1:
            _attention_kernel(tc, out, xT, wqT, wkT, wvT, ident, kpos, qpos, phase)
        else:
            with tc.For_i(0, iters, 1):
                _attention_kernel(
                    tc, out, xT, wqT, wkT, wvT, ident, kpos, qpos, phase
                )
    nc.compile()
    return nc


def _attention_kernel(tc, out, xT, wqT, wkT, wvT, ident, kpos, qpos, phase):
    nc = tc.nc
    with ExitStack() as ctx:
        # ---- inputs resident in SBUF (single big tiles) ----
        in_pool = ctx.enter_context(tc.tile_pool(name="inp", bufs=1))
        w_pool = ctx.enter_context(tc.tile_pool(name="w", bufs=2))
        xp = in_pool.tile([P, W8], BF16, tag="xp")
        nc.sync.dma_start(xp[:], xT[:, :])
        wk = w_pool.tile([P, W8], BF16, tag="w", name="wk")
        nc.sync.dma_start(wk[:], wkT[:, :])
        wv = w_pool.tile([P, W8], BF16, tag="w", name="wv")
        nc.sync.dma_start(wv[:], wvT[:, :])

        # ---- DRAM bounce buffers for the pairwise K/V all-gathers ----
        dram = ctx.enter_context(tc.tile_pool(name="dram", bufs=1, space="DRAM"))
        k_g = dram.tile([2 * P, W8], BF16, tag="k_g")
        v_g = dram.tile([2 * P, W8], BF16, tag="v_g")
        if phase != "nocoll":
            k_own = dram.tile([P, W8], BF16, tag="k_own")
            v_own = dram.tile([P, W8], BF16, tag="v_own")

        # ---- persistent result tiles ----
        big_pool = ctx.enter_context(tc.tile_pool(name="big", bufs=1))
        QT = big_pool.tile([P, W8], BF16, tag="qt")       # [o, ob*1024 + q]
        KT = big_pool.tile([P, 2 * W8], BF16, tag="kt")   # [A|B], ob-major
        Vt = big_pool.tile([P, 2 * W8], BF16, tag="vt")   # [A|B], tile-major
        KS = big_pool.tile([P, W8], BF16, tag="ks")       # K staging
        VS = big_pool.tile([P, W8], BF16, tag="vs")       # V staging

        # ================= projections (own 1024 rows only) =================
        with ExitStack() as pctx:
            psum_p = pctx.enter_context(tc.tile_pool(name="psum_p", bufs=4, space="PSUM"))

            # --- K: SG[p, ob*1024 + k] = K^T[ob*128+p, k] ---
            for ob in range(8):
                ps = [psum_p.tile([P, 512], F32, tag="pp", name=f"kp{c}") for c in range(2)]
                for d in range(8):
                    for c in range(2):
                        nc.tensor.matmul(
                            ps[c][:],
                            wk[:, d * 1024 + ob * P : d * 1024 + (ob + 1) * P],
                            xp[:, d * 1024 + c * 512 : d * 1024 + (c + 1) * 512],
                            start=(d == 0),
                            stop=(d == 7),
                        )
                for c in range(2):
                    nc.scalar.copy(
                        KS[:, ob * 1024 + c * 512 : ob * 1024 + (c + 1) * 512],
                        ps[c][:],
                    )
            # wq shares wk's buffer; load as soon as K proj releases it
            wq = w_pool.tile([P, W8], BF16, tag="w", name="wq")
            nc.sync.dma_start(wq[:], wqT[:, :])
            if phase != "nocoll":
                nc.sync.dma_start(k_own[:, :], KS[:])
                nc.gpsimd.collective_compute(
                    "AllGather",
                    mybir.AluOpType.bypass,
                    replica_groups=GROUPS,
                    ins=[k_own.opt()],
                    outs=[k_g.opt()],
                )
            else:
                # timing variant: own data stands in for both gathered halves
                nc.sync.dma_start(k_g[0:P, :], KS[:])
                nc.sync.dma_start(k_g[P : 2 * P, :], KS[:])
            # gathered K loads (overlap the V/Q projections)
            nc.sync.dma_start(KT[:, 0:W8], k_g[0:P, :])
            nc.sync.dma_start(KT[:, W8 : 2 * W8], k_g[P : 2 * P, :])

            # --- V: SG[p, t*1024 + o] = V[t*128+p, o] ---
            for t in range(8):
                ps = [psum_p.tile([P, 512], F32, tag="pp", name=f"vp{c}") for c in range(2)]
                for d in range(8):
                    for c in range(2):
                        nc.tensor.matmul(
                            ps[c][:],
                            xp[:, d * 1024 + t * P : d * 1024 + (t + 1) * P],
                            wv[:, d * 1024 + c * 512 : d * 1024 + (c + 1) * 512],
                            start=(d == 0),
                            stop=(d == 7),
                        )
                for c in range(2):
                    nc.scalar.copy(
                        VS[:, t * 1024 + c * 512 : t * 1024 + (c + 1) * 512],
                        ps[c][:],
                    )
            if phase != "nocoll":
                nc.sync.dma_start(v_own[:, :], VS[:])
                nc.gpsimd.collective_compute(
                    "AllGather",
                    mybir.AluOpType.bypass,
                    replica_groups=GROUPS,
                    ins=[v_own.opt()],
                    outs=[v_g.opt()],
                )
            else:
                nc.sync.dma_start(v_g[0:P, :], VS[:])
                nc.sync.dma_start(v_g[P : 2 * P, :], VS[:])
            # gathered V loads (overlap the Q projection)
            nc.sync.dma_start(Vt[:, 0:W8], v_g[0:P, :])
            nc.sync.dma_start(Vt[:, W8 : 2 * W8], v_g[P : 2 * P, :])

            # --- Q: QT[p, ob*1024 + q] stays in SBUF ---
            for ob in range(8):
                ps = [psum_p.tile([P, 512], F32, tag="pp", name=f"qp{c}") for c in range(2)]
                for d in range(8):
                    for c in range(2):
                        nc.tensor.matmul(
                            ps[c][:],
                            wq[:, d * 1024 + ob * P : d * 1024 + (ob + 1) * P],
                            xp[:, d * 1024 + c * 512 : d * 1024 + (c + 1) * 512],
                            start=(d == 0),
                            stop=(d == 7),
                        )
                for c in range(2):
                    nc.scalar.copy(
                        QT[:, ob * 1024 + c * 512 : ob * 1024 + (c + 1) * 512],
                        ps[c][:],
                    )

        # ================= attention =================
        p_pool = ctx.enter_context(tc.tile_pool(name="p", bufs=2))
        m_pool = ctx.enter_context(tc.tile_pool(name="m", bufs=2))
        stmp_pool = ctx.enter_context(tc.tile_pool(name="stmp", bufs=2))
        acc_pool = ctx.enter_context(tc.tile_pool(name="acc", bufs=24))
        pt_pool = ctx.enter_context(tc.tile_pool(name="pt", bufs=4))
        o_pool = ctx.enter_context(tc.tile_pool(name="o", bufs=2))
        psum_s = ctx.enter_context(tc.tile_pool(name="psum_s", bufs=3, space="PSUM"))
        psum_t = ctx.enter_context(tc.tile_pool(name="psum_t", bufs=2, space="PSUM"))
        psum_o = ctx.enter_context(tc.tile_pool(name="psum_o", bufs=2, space="PSUM"))

        for j in range(7, -1, -1):
            w = (j + 1) * P
            vis = j * P
            Pt = p_pool.tile([P, N], BF16, tag="p")
            accs = []
            for half in range(2):
                kbase = half * NQ      # gathered key index base (for kpos)
                mbase = half * W8      # KT column base
                ppos = half * w        # packed col base in Pt
                M = m_pool.tile([P, P], F32, tag="m")
                nc.vector.tensor_scalar(
                    M[:],
                    kpos[:, kbase + vis : kbase + w],
                    qpos[:, j : j + 1],
                    MASK_VAL,
                    op0=mybir.AluOpType.is_gt,
                    op1=mybir.AluOpType.mult,
                )
                for (c0, c1) in _vis_chunks(vis):
                    cw = c1 - c0
                    ps = psum_s.tile([P, 512], F32, tag="ps")
                    for ob in range(8):
                        nc.tensor.matmul(
                            ps[:, 0:cw],
                            QT[:, ob * 1024 + j * P : ob * 1024 + (j + 1) * P],
                            KT[:, mbase + ob * 1024 + c0 : mbase + ob * 1024 + c1],
                            start=(ob == 0),
                            stop=(ob == 7),
                        )
                    acc = acc_pool.tile([P, 1], F32, tag="acc")
                    nc.scalar.activation(
                        Pt[:, ppos + c0 : ppos + c1],
                        ps[:, 0:cw],
                        mybir.ActivationFunctionType.Exp,
                        accum_out=acc[:],
                    )
                    accs.append(acc)
                # candidate (diagonal / padded) tile
                ps = psum_s.tile([P, 512], F32, tag="ps")
                for ob in range(8):
                    nc.tensor.matmul(
                        ps[:, 0:P],
                        QT[:, ob * 1024 + j * P : ob * 1024 + (j + 1) * P],
                        KT[:, mbase + ob * 1024 + vis : mbase + ob * 1024 + w],
                        start=(ob == 0),
                        stop=(ob == 7),
                    )
                st = stmp_pool.tile([P, P], F32, tag="stmp")
                nc.vector.tensor_tensor(st[:], ps[:, 0:P], M[:], mybir.AluOpType.add)
                acc = acc_pool.tile([P, 1], F32, tag="acc")
                nc.scalar.activation(
                    Pt[:, ppos + vis : ppos + w],
                    st[:],
                    mybir.ActivationFunctionType.Exp,
                    accum_out=acc[:],
                )
                accs.append(acc)

            dn = acc_pool.tile([P, 1], F32, tag="dn")
            nc.vector.tensor_tensor(dn[:], accs[0][:], accs[1][:], mybir.AluOpType.add)
            for a in accs[2:]:
                nc.vector.tensor_tensor(dn[:], dn[:], a[:], mybir.AluOpType.add)
            rz = acc_pool.tile([P, 1], F32, tag="rz")
            nc.vector.reciprocal(rz[:], dn[:])

            # ---- P^T tiles + AV (transposes pipelined one tile ahead) ----
            ntile = 2 * (j + 1)
            ops = [psum_o.tile([P, 512], F32, tag="po", name=f"o{c}") for c in range(2)]
            pts = {}

            def do_tr(t, Pt=Pt, pts=pts):
                tp = psum_t.tile([P, P], BF16, tag="pt")
                nc.tensor.transpose(tp[:], Pt[:, t * P : (t + 1) * P], ident[:])
                sb = pt_pool.tile([P, P], BF16, tag="ptsb")
                nc.vector.tensor_copy(sb[:], tp[:])
                pts[t] = sb

            do_tr(0)
            for t in range(ntile):
                if t + 1 < ntile:
                    do_tr(t + 1)
                gt = t if t <= j else 8 + (t - (j + 1))
                vbase = (gt % 8) * 1024 + (gt // 8) * W8
                for c in range(2):
                    nc.tensor.matmul(
                        ops[c][:],
                        pts[t][:],
                        Vt[:, vbase + c * 512 : vbase + (c + 1) * 512],
                        start=(t == 0),
                        stop=(t == ntile - 1),
                    )
            O = o_pool.tile([P, D], BF16, tag="o")
            for c in range(2):
                nc.vector.tensor_scalar_mul(O[:, c * 512 : (c + 1) * 512], ops[c][:], rz[:])
            # store via the idle Pool engine's DGE queue: keeps the SP queue
            # free so the next iteration's input loads prefetch during this
            # iteration's attention
            nc.gpsimd.dma_start(out[j * P : (j + 1) * P, :], O[:])


def _get_program(iters=1, phase="full"):
    key = ("nc", iters, phase)
    if key not in _CACHE:
        _CACHE[key] = _build_program(iters, phase)
    return _CACHE[key]


def _interleave(mT):
    """[1024, 1024] row-major (d, o) -> [128, 8192] with col = d*1024 + o."""
    return np.ascontiguousarray(
        mT.reshape(8, P, 1024).transpose(1, 0, 2).reshape(P, W8)
    )


def _host_prep(x, Wq, Wk, Wv):
    bf = ml_dtypes.bfloat16
    scale = np.float32(1.0 / np.sqrt(np.float32(D)))
    wqT = _interleave((np.asarray(Wq, np.float32) * scale).T).astype(bf)
    wkT = _interleave(np.asarray(Wk, np.float32).T).astype(bf)
    wvT = _interleave(np.asarray(Wv, np.float32).T).astype(bf)
    ident = np.eye(P, dtype=np.float32).astype(bf)
    kpos = np.tile(
        np.concatenate([np.arange(g * P, (g + 1) * P) for g in GATHERED])
        .astype(np.float32)[None, :],
        (P, 1),
    )
    parange = np.arange(P, dtype=np.float32)
    in_maps = []
    for c in range(8):
        b, h = c // 2, c % 2
        blocks = ASC_A if h == 0 else ASC_B
        rows = np.concatenate(
            [np.asarray(x[b, g * P : (g + 1) * P], np.float32) for g in blocks], 0
        )
        xTl = _interleave(np.ascontiguousarray(rows.T)).astype(bf)
        qpos = np.empty((P, NB), np.float32)
        for jj, g in enumerate(blocks):
            qpos[:, jj] = g * P + parange
        in_maps.append(
            {
                "xT": xTl,
                "wqT": wqT,
                "wkT": wkT,
                "wvT": wvT,
                "kpos": kpos,
                "qpos": qpos,
                "ident": ident,
            }
        )
    return in_maps


def unshard(core_outs):
    out = np.empty((B, N, D), np.float32)
    for c in range(8):
        b, h = c // 2, c % 2
        blocks = ASC_A if h == 0 else ASC_B
        res = np.asarray(core_outs[c], np.float32)
        for jj, g in enumerate(blocks):
            out[b, g * P : (g + 1) * P] = res[jj * P : (jj + 1) * P]
    return out


def kernel(x, Wq, Wk, Wv):
    nc = _get_program()
    in_maps = _host_prep(x, Wq, Wk, Wv)
    res = run_bass_kernel_spmd(nc, in_maps, list(range(8)))
    _CACHE["last_results"] = res
    return unshard([res.results[c]["out"] for c in range(8)])


# revision 22
# speedup vs baseline: 1.0822x; 1.0822x over previous
"""Causal single-head attention on 8 Trainium2 NeuronCores (Bass/Tile), v4.

Problem: x[4,2048,1024] fp32, Wq/Wk/Wv[1024,1024];
  q,k,v = x@W.T ; S = q@k.T/sqrt(d) ; causal softmax ; out = P@v.

Sharding (balanced causal): core c -> batch b=c//2, half h=c%2. Each core
owns 8 query blocks of 128 interleaved so causal work balances:
  h=0: global q-blocks [0,3,4,7,8,11,12,15]   (key extents 1,4,5,8,9,12,13,16)
  h=1: global q-blocks [1,2,5,6,9,10,13,14]   (extents 2,3,6,7,10,11,14,15)
Each core projects Q/K/V only for its own 1024 rows; K/V halves are exchanged
with a pairwise AllGather (groups [[0,1],[2,3],[4,5],[6,7]]), giving both
cores the full K/V in "gathered" key order [A-blocks asc | B-blocks asc].

Attention slot j (local q-block j, processed j=7..0) computes scores only
over the causally-needed prefix: the first j+1 gathered tiles of EACH half
(<=1 padded tile per slot; 72 of 128 tiles computed per core). Only the
last tile of each half can be partial/masked; the mask is built data-driven
from kpos/qpos (SPMD-uniform program, per-core data).

All SBUF-resident operands use block-interleaved [128, n*1024] layouts so
every bulk transfer is a single large DMA (the HW cost is dominated by
per-DMA overhead at these sizes): x/W/Q/K-stage are one tile each; the
gathered K^T and V are [128, 16384] tiles split [A-half | B-half], each
half ob-/tile-major so score and AV moving slices stay contiguous.

Numerics: bf16 operands with fp32 PSUM accumulation; softmax without
max-subtraction (scores are O(+-6)); denominator via activation accum_out;
P@V via PE-transposed P tiles; bf16 output upcast on host. End-to-end rel
err ~5e-3 (gate 2e-2).
"""

import sys

sys.path.insert(0, "/opt/trn_rl_repo")

from contextlib import ExitStack

import numpy as np
import ml_dtypes

import concourse.bass as bass  # noqa: F401
from concourse import bacc
import concourse.mybir as mybir
import concourse.tile as tile
from concourse.bass_utils import run_bass_kernel_spmd

F32 = mybir.dt.float32
BF16 = mybir.dt.bfloat16

B, N, D = 4, 2048, 1024
P = 128
NQ = 1024        # local queries per core
NB = 8           # local q-blocks per core
W8 = 8 * 1024    # interleaved big-tile width
ASC_A = [0, 3, 4, 7, 8, 11, 12, 15]
ASC_B = [1, 2, 5, 6, 9, 10, 13, 14]
GATHERED = ASC_A + ASC_B
MASK_VAL = -1.0e30
GROUPS = [[0, 1], [2, 3], [4, 5], [6, 7]]

_CACHE = {}


def _vis_chunks(w):
    """Split visible width w (multiple of 128, <=896) into chunks <=512,
    avoiding a trailing 128-wide chunk where possible."""
    out, o, rem = [], 0, w
    while rem > 0:
        c = 512 if rem >= 512 else rem
        if 512 < rem < 768:
            c = rem - 256
        out.append((o, o + c))
        o += c
        rem -= c
    return out


def _build_program(iters=1, phase="full"):
    nc = bacc.Bacc("TRN2", target_bir_lowering=False, debug=False, num_devices=8)
    xT = nc.dram_tensor("xT", [P, W8], BF16, kind="ExternalInput").ap()
    wqT = nc.dram_tensor("wqT", [P, W8], BF16, kind="ExternalInput").ap()
    wkT = nc.dram_tensor("wkT", [P, W8], BF16, kind="ExternalInput").ap()
    wvT = nc.dram_tensor("wvT", [P, W8], BF16, kind="ExternalInput").ap()
    kposd = nc.dram_tensor("kpos", [P, N], F32, kind="ExternalInput").ap()
    qposd = nc.dram_tensor("qpos", [P, NB], F32, kind="ExternalInput").ap()
    identd = nc.dram_tensor("ident", [P, P], BF16, kind="ExternalInput").ap()
    out = nc.dram_tensor("out", [NQ, D], BF16, kind="ExternalOutput").ap()

    with tile.TileContext(nc) as tc, ExitStack() as cstack:
        # loop-invariant constants (loaded once per launch, like the
        # single-shot kernel)
        const = cstack.enter_context(tc.tile_pool(name="const", bufs=1))
        ident = const.tile([P, P], BF16, tag="ident")
        nc.sync.dma_start(ident[:], identd[:, :])
        kpos = const.tile([P, N], F32, tag="kpos")
        nc.sync.dma_start(kpos[:], kposd[:, :])
        qpos = const.tile([P, NB], F32, tag="qpos")
        nc.sync.dma_start(qpos[:], qposd[:, :])

        if iters == 1:
            _attention_kernel(tc, out, xT, wqT, wkT, wvT, ident, kpos, qpos, phase)
        else:
            with tc.For_i(0, iters, 1):
                _attention_kernel(
                    tc, out, xT, wqT, wkT, wvT, ident, kpos, qpos, phase
                )
    nc.compile()
    return nc


def _attention_kernel(tc, out, xT, wqT, wkT, wvT, ident, kpos, qpos, phase):
    nc = tc.nc
    with ExitStack() as ctx:
        # ---- inputs resident in SBUF (single big tiles) ----
        in_pool = ctx.enter_context(tc.tile_pool(name="inp", bufs=1))
        w_pool = ctx.enter_context(tc.tile_pool(name="w", bufs=2))
        xp = in_pool.tile([P, W8], BF16, tag="xp")
        nc.sync.dma_start(xp[:], xT[:, :])
        wk = w_pool.tile([P, W8], BF16, tag="w", name="wk")
        nc.sync.dma_start(wk[:], wkT[:, :])
        wv = w_pool.tile([P, W8], BF16, tag="w", name="wv")
        nc.sync.dma_start(wv[:], wvT[:, :])

        # ---- DRAM bounce buffers for the pairwise K/V all-gathers ----
        dram = ctx.enter_context(tc.tile_pool(name="dram", bufs=1, space="DRAM"))
        k_g = dram.tile([2 * P, W8], BF16, tag="k_g")
        v_g = dram.tile([2 * P, W8], BF16, tag="v_g")
        if phase != "nocoll":
            k_own = dram.tile([P, W8], BF16, tag="k_own")
            v_own = dram.tile([P, W8], BF16, tag="v_own")

        # ---- persistent result tiles ----
        big_pool = ctx.enter_context(tc.tile_pool(name="big", bufs=1))
        QT = big_pool.tile([P, W8], BF16, tag="qt")       # [o, ob*1024 + q]
        KT = big_pool.tile([P, 2 * W8], BF16, tag="kt")   # [A|B], ob-major
        Vt = big_pool.tile([P, 2 * W8], BF16, tag="vt")   # [A|B], tile-major
        KS = big_pool.tile([P, W8], BF16, tag="ks")       # K staging
        VS = big_pool.tile([P, W8], BF16, tag="vs")       # V staging

        # ================= projections (own 1024 rows only) =================
        with ExitStack() as pctx:
            psum_p = pctx.enter_context(tc.tile_pool(name="psum_p", bufs=4, space="PSUM"))

            # --- K: SG[p, ob*1024 + k] = K^T[ob*128+p, k] ---
            for ob in range(8):
                ps = [psum_p.tile([P, 512], F32, tag="pp", name=f"kp{c}") for c in range(2)]
                for d in range(8):
                    for c in range(2):
                        nc.tensor.matmul(
                            ps[c][:],
                            wk[:, d * 1024 + ob * P : d * 1024 + (ob + 1) * P],
                            xp[:, d * 1024 + c * 512 : d * 1024 + (c + 1) * 512],
                            start=(d == 0),
                            stop=(d == 7),
                        )
                for c in range(2):
                    nc.scalar.copy(
                        KS[:, ob * 1024 + c * 512 : ob * 1024 + (c + 1) * 512],
                        ps[c][:],
                    )
            # wq shares wk's buffer; load as soon as K proj releases it
            wq = w_pool.tile([P, W8], BF16, tag="w", name="wq")
            nc.sync.dma_start(wq[:], wqT[:, :])
            if phase != "nocoll":
                nc.sync.dma_start(k_own[:, :], KS[:])
                nc.gpsimd.collective_compute(
                    "AllGather",
                    mybir.AluOpType.bypass,
                    replica_groups=GROUPS,
                    ins=[k_own.opt()],
                    outs=[k_g.opt()],
                )
            else:
                # timing variant: own data stands in for both gathered halves
                nc.sync.dma_start(k_g[0:P, :], KS[:])
                nc.sync.dma_start(k_g[P : 2 * P, :], KS[:])
            # gathered K loads (overlap the V/Q projections)
            nc.sync.dma_start(KT[:, 0:W8], k_g[0:P, :])
            nc.sync.dma_start(KT[:, W8 : 2 * W8], k_g[P : 2 * P, :])

            # --- V: SG[p, t*1024 + o] = V[t*128+p, o] ---
            for t in range(8):
                ps = [psum_p.tile([P, 512], F32, tag="pp", name=f"vp{c}") for c in range(2)]
                for d in range(8):
                    for c in range(2):
                        nc.tensor.matmul(
                            ps[c][:],
                            xp[:, d * 1024 + t * P : d * 1024 + (t + 1) * P],
                            wv[:, d * 1024 + c * 512 : d * 1024 + (c + 1) * 512],
                            start=(d == 0),
                            stop=(d == 7),
                        )
                for c in range(2):
                    nc.scalar.copy(
                        VS[:, t * 1024 + c * 512 : t * 1024 + (c + 1) * 512],
                        ps[c][:],
                    )
            if phase != "nocoll":
                nc.sync.dma_start(v_own[:, :], VS[:])
                nc.gpsimd.collective_compute(
                    "AllGather",
                    mybir.AluOpType.bypass,
                    replica_groups=GROUPS,
                    ins=[v_own.opt()],
                    outs=[v_g.opt()],
                )
            else:
                nc.sync.dma_start(v_g[0:P, :], VS[:])
                nc.sync.dma_start(v_g[P : 2 * P, :], VS[:])
            # gathered V loads (overlap the Q projection)
            nc.sync.dma_start(Vt[:, 0:W8], v_g[0:P, :])
            nc.sync.dma_start(Vt[:, W8 : 2 * W8], v_g[P : 2 * P, :])

            # --- Q: QT[p, ob*1024 + q] stays in SBUF ---
            for ob in range(8):
                ps = [psum_p.tile([P, 512], F32, tag="pp", name=f"qp{c}") for c in range(2)]
                for d in range(8):
                    for c in range(2):
                        nc.tensor.matmul(
                            ps[c][:],
                            wq[:, d * 1024 + ob * P : d * 1024 + (ob + 1) * P],
                            xp[:, d * 1024 + c * 512 : d * 1024 + (c + 1) * 512],
                            start=(d == 0),
                            stop=(d == 7),
                        )
                for c in range(2):
                    nc.scalar.copy(
                        QT[:, ob * 1024 + c * 512 : ob * 1024 + (c + 1) * 512],
                        ps[c][:],
                    )

        # ================= attention =================
        p_pool = ctx.enter_context(tc.tile_pool(name="p", bufs=2))
        m_pool = ctx.enter_context(tc.tile_pool(name="m", bufs=2))
        stmp_pool = ctx.enter_context(tc.tile_pool(name="stmp", bufs=2))
        acc_pool = ctx.enter_context(tc.tile_pool(name="acc", bufs=24))
        pt_pool = ctx.enter_context(tc.tile_pool(name="pt", bufs=4))
        o_pool = ctx.enter_context(tc.tile_pool(name="o", bufs=2))
        psum_s = ctx.enter_context(tc.tile_pool(name="psum_s", bufs=4, space="PSUM"))
        psum_t = ctx.enter_context(tc.tile_pool(name="psum_t", bufs=2, space="PSUM"))
        psum_o = ctx.enter_context(tc.tile_pool(name="psum_o", bufs=2, space="PSUM"))

        for j in range(7, -1, -1):
            w = (j + 1) * P
            vis = j * P
            Pt = p_pool.tile([P, N], BF16, tag="p")
            accs = []
            for half in range(2):
                kbase = half * NQ      # gathered key index base (for kpos)
                mbase = half * W8      # KT column base
                ppos = half * w        # packed col base in Pt
                M = m_pool.tile([P, P], F32, tag="m")
                nc.vector.tensor_scalar(
                    M[:],
                    kpos[:, kbase + vis : kbase + w],
                    qpos[:, j : j + 1],
                    MASK_VAL,
                    op0=mybir.AluOpType.is_gt,
                    op1=mybir.AluOpType.mult,
                )
                for (c0, c1) in _vis_chunks(vis):
                    cw = c1 - c0
                    ps = psum_s.tile([P, 512], F32, tag="ps")
                    for ob in range(8):
                        nc.tensor.matmul(
                            ps[:, 0:cw],
                            QT[:, ob * 1024 + j * P : ob * 1024 + (j + 1) * P],
                            KT[:, mbase + ob * 1024 + c0 : mbase + ob * 1024 + c1],
                            start=(ob == 0),
                            stop=(ob == 7),
                        )
                    acc = acc_pool.tile([P, 1], F32, tag="acc")
                    nc.scalar.activation(
                        Pt[:, ppos + c0 : ppos + c1],
                        ps[:, 0:cw],
                        mybir.ActivationFunctionType.Exp,
                        accum_out=acc[:],
                    )
                    accs.append(acc)
                # candidate (diagonal / padded) tile
                ps = psum_s.tile([P, 512], F32, tag="ps")
                for ob in range(8):
                    nc.tensor.matmul(
                        ps[:, 0:P],
                        QT[:, ob * 1024 + j * P : ob * 1024 + (j + 1) * P],
                        KT[:, mbase + ob * 1024 + vis : mbase + ob * 1024 + w],
                        start=(ob == 0),
                        stop=(ob == 7),
                    )
                st = stmp_pool.tile([P, P], F32, tag="stmp")
                nc.vector.tensor_tensor(st[:], ps[:, 0:P], M[:], mybir.AluOpType.add)
                acc = acc_pool.tile([P, 1], F32, tag="acc")
                nc.scalar.activation(
                    Pt[:, ppos + vis : ppos + w],
                    st[:],
                    mybir.ActivationFunctionType.Exp,
                    accum_out=acc[:],
                )
                accs.append(acc)

            dn = acc_pool.tile([P, 1], F32, tag="dn")
            nc.vector.tensor_tensor(dn[:], accs[0][:], accs[1][:], mybir.AluOpType.add)
            for a in accs[2:]:
                nc.vector.tensor_tensor(dn[:], dn[:], a[:], mybir.AluOpType.add)
            rz = acc_pool.tile([P, 1], F32, tag="rz")
            nc.vector.reciprocal(rz[:], dn[:])

            # ---- P^T tiles + AV (transposes pipelined one tile ahead) ----
            ntile = 2 * (j + 1)
            ops = [psum_o.tile([P, 512], F32, tag="po", name=f"o{c}") for c in range(2)]
            pts = {}

            def do_tr(t, Pt=Pt, pts=pts):
                tp = psum_t.tile([P, P], BF16, tag="pt")
                nc.tensor.transpose(tp[:], Pt[:, t * P : (t + 1) * P], ident[:])
                sb = pt_pool.tile([P, P], BF16, tag="ptsb")
                # evict on the scalar engine: DVE reads from PSUM are the
                # prime suspect for the HW-vs-sim gap
                nc.scalar.copy(sb[:], tp[:])
                pts[t] = sb

            do_tr(0)
            for t in range(ntile):
                if t + 1 < ntile:
                    do_tr(t + 1)
                gt = t if t <= j else 8 + (t - (j + 1))
                vbase = (gt % 8) * 1024 + (gt // 8) * W8
                for c in range(2):
                    nc.tensor.matmul(
                        ops[c][:],
                        pts[t][:],
                        Vt[:, vbase + c * 512 : vbase + (c + 1) * 512],
                        start=(t == 0),
                        stop=(t == ntile - 1),
                    )
            O = o_pool.tile([P, D], BF16, tag="o")
            for c in range(2):
                nc.scalar.mul(O[:, c * 512 : (c + 1) * 512], ops[c][:], rz[:])
            # store via the idle Pool engine's DGE queue: keeps the SP queue
            # free so the next iteration's input loads prefetch during this
            # iteration's attention
            nc.gpsimd.dma_start(out[j * P : (j + 1) * P, :], O[:])


def _get_program(iters=1, phase="full"):
    key = ("nc", iters, phase)
    if key not in _CACHE:
        _CACHE[key] = _build_program(iters, phase)
    return _CACHE[key]


def _interleave(mT):
    """[1024, 1024] row-major (d, o) -> [128, 8192] with col = d*1024 + o."""
    return np.ascontiguousarray(
        mT.reshape(8, P, 1024).transpose(1, 0, 2).reshape(P, W8)
    )


def _host_prep(x, Wq, Wk, Wv):
    bf = ml_dtypes.bfloat16
    scale = np.float32(1.0 / np.sqrt(np.float32(D)))
    wqT = _interleave((np.asarray(Wq, np.float32) * scale).T).astype(bf)
    wkT = _interleave(np.asarray(Wk, np.float32).T).astype(bf)
    wvT = _interleave(np.asarray(Wv, np.float32).T).astype(bf)
    ident = np.eye(P, dtype=np.float32).astype(bf)
    kpos = np.tile(
        np.concatenate([np.arange(g * P, (g + 1) * P) for g in GATHERED])
        .astype(np.float32)[None, :],
        (P, 1),
    )
    parange = np.arange(P, dtype=np.float32)
    in_maps = []
    for c in range(8):
        b, h = c // 2, c % 2
        blocks = ASC_A if h == 0 else ASC_B
        rows = np.concatenate(
            [np.asarray(x[b, g * P : (g + 1) * P], np.float32) for g in blocks], 0
        )
        xTl = _interleave(np.ascontiguousarray(rows.T)).astype(bf)
        qpos = np.empty((P, NB), np.float32)
        for jj, g in enumerate(blocks):
            qpos[:, jj] = g * P + parange
        in_maps.append(
            {
                "xT": xTl,
                "wqT": wqT,
                "wkT": wkT,
                "wvT": wvT,
                "kpos": kpos,
                "qpos": qpos,
                "ident": ident,
            }
        )
    return in_maps


def unshard(core_outs):
    out = np.empty((B, N, D), np.float32)
    for c in range(8):
        b, h = c // 2, c % 2
        blocks = ASC_A if h == 0 else ASC_B
        res = np.asarray(core_outs[c], np.float32)
        for jj, g in enumerate(blocks):
            out[b, g * P : (g + 1) * P] = res[jj * P : (jj + 1) * P]
    return out


def kernel(x, Wq, Wk, Wv):
    nc = _get_program()
    in_maps = _host_prep(x, Wq, Wk, Wv)
    res = run_bass_kernel_spmd(nc, in_maps, list(range(8)))
    _CACHE["last_results"] = res
    return unshard([res.results[c]["out"] for c in range(8)])
